# revision 1
# baseline (speedup 1.0000x reference)
"""MoE segment-gated rank-1 LoRA projection for Trainium2 (8 NeuronCores).

Math: out[b,s,:] = sum_k topk_score[b,k] * SCALE * (x[b,s,:]@A[e_k]) * B[e_k]
Since gating is per-batch (segment level), this is, per batch b:
    H^T[e, t] = A[e, :] @ x[b]^T          (contract IN=1024)
    out[b]^T  = M2[b]^T.T @ H^T           (contract E=8)
where M2[b][e, :] = g[b, e] * SCALE * B[e, :], g zero for unselected experts.

Sharding: 8 cores <- (batch b = c//2, seq half h = c%2); each core owns 2048
tokens: streams x^T in (8MB), writes out^T (8MB). Host does the tiny gating
([4,8] softmax/top-2) and the x transposes.
"""

import numpy as np

import concourse.bass as bass
import concourse.tile as tile
from concourse import bacc, mybir
from concourse.bass_utils import run_bass_kernel_spmd

B, S, IN, OUT, E = 4, 4096, 1024, 1024, 8
TOPK = 2
SCALE = 512.0
TEMP = 1.0
N_CORES = 8
T = (B * S) // N_CORES          # 2048 tokens per core
P = 128
KT = IN // P                    # 8 contraction tiles
OTILES = OUT // P               # 8 output row-tiles

# Token chunk schedule: small chunks at the start (PE starts after one small
# DMA wave instead of a full 512-token wave) and at the end (the last stores
# + drain shrink); big chunks in the middle for low per-instruction overhead.
CHUNKS = [512, 512, 512, 512]
assert sum(CHUNKS) == T
NCHUNK = len(CHUNKS)

# Matmul operand dtype: "f32" (exact, 4cyc/row), "f32r" (TF32-ish, ~2cyc/row),
# "bf16" (1cyc/row at 2.4GHz + halves x DMA traffic).
MM_DTYPE = "bf16"
# Output dtype on the wire: bf16 halves store traffic; host upcasts to f32.
OUT_BF16 = True

_NC = None


def _build_bass():
    # Bacc (not raw Bass): its compile() pass splits multi-sem-waits into
    # EventSemaphore instructions — TRN2 instructions fit only one wait.
    nc = bacc.Bacc()
    dt_mm = {"f32": mybir.dt.float32,
             "f32r": mybir.dt.float32r,
             "bf16": mybir.dt.bfloat16}[MM_DTYPE]
    xT = nc.dram_tensor("xT", [IN, T], dt_mm, kind="ExternalInput")
    aT = nc.dram_tensor("aT", [P, KT * E], dt_mm, kind="ExternalInput")
    m2 = nc.dram_tensor("m2", [E, OUT], dt_mm, kind="ExternalInput")
    dt_out = mybir.dt.bfloat16 if OUT_BF16 else mybir.dt.float32
    outT = nc.dram_tensor("outT", [OUT, T], dt_out, kind="ExternalOutput")

    xT_k = xT.rearrange("(k p) t -> k p t", p=P)      # [KT, 128, T]
    outT_k = outT.rearrange("(o p) t -> o p t", p=P)  # [OTILES, 128, T]

    # Bacc splits multi-sem waits, so no manual wait engineering is needed.
    # Loads are issued alternately from SP (nc.sync) and ACT (nc.scalar) HWDGE
    # sequencers (~660ns issue cost each); stores from Pool (gpsimd, SWDGE).
    with tile.TileContext(nc) as tc:
        with (
            tc.tile_pool(name="consts", bufs=1) as consts,
            tc.tile_pool(name="xin", bufs=NCHUNK) as xin,
            tc.tile_pool(name="hbuf", bufs=3) as hbuf,
            tc.tile_pool(name="obuf", bufs=4) as obuf,
            tc.tile_pool(name="psh", bufs=2, space="PSUM") as psh,
            tc.tile_pool(name="pso", bufs=5, space="PSUM") as pso,
            tc.tile_pool(name="warm", bufs=1, space="PSUM") as warm,
        ):
            a_sb = consts.tile([P, KT * E], dt_mm)
            nc.sync.dma_start(a_sb[:], aT[:])
            wsrc = consts.tile([P, 512], dt_mm)
            nc.vector.memset(wsrc[:], 0.0)
            wsink = consts.tile([P, 4], mybir.dt.float32)
            m2_sb = consts.tile([E, OUT], dt_mm)
            nc.scalar.dma_start(m2_sb[:], m2[:])

            tok_of = []
            base = 0
            for c in range(NCHUNK):
                tok_of.append(slice(base, base + CHUNKS[c]))
                base += CHUNKS[c]

            def emit_stage1(c):
                """loads + matmul1 + h copy for chunk c; returns h tile."""
                CH = CHUNKS[c]
                tok = tok_of[c]
                xks = []
                for k in range(KT):
                    xk = xin.tile([P, CH], dt_mm, tag=f"x{k}")
                    # spread loads over HWDGE (sync) and SWDGE (gpsimd):
                    # 16 hw queues total carry the traffic
                    eng = nc.sync if k % 2 == 0 else nc.gpsimd
                    eng.dma_start(xk[:], xT_k[k, :, tok])
                    xks.append(xk)
                ph = psh.tile([E, CH], mybir.dt.float32)
                for k in range(KT):
                    nc.tensor.matmul(
                        ph[:],
                        a_sb[:, k * E:(k + 1) * E],
                        xks[k][:],
                        start=(k == 0),
                        stop=(k == KT - 1),
                    )
                h = hbuf.tile([E, CH], dt_mm)
                # h copy on ACT keeps DVE free for the output casts
                nc.scalar.copy(h[:], ph[:])
                # one full-array (128x128) matmul per chunk keeps the HAM
                # activity monitor fed: with only skinny (8-row/8-col) real
                # matmuls the clock gate throttles PE to 1.2GHz
                wt = warm.tile([P, 512], mybir.dt.float32)
                nc.tensor.matmul(wt[:], wsrc[:, 0:P], wsrc[:],
                                 start=True, stop=True)
                nc.vector.tensor_copy(wsink[:], wt[:, 0:4])
                return h

            def emit_stage2(c, h):
                """matmul2 + output cast + store for chunk c. The last chunk
                runs in half-size token groups so its final stores drain in
                half the time."""
                CH = CHUNKS[c]
                tok = tok_of[c]
                splits = [(0, CH)]
                for lo, hi in splits:
                    for o in range(OTILES):
                        po = pso.tile([P, hi - lo], mybir.dt.float32)
                        nc.tensor.matmul(
                            po[:],
                            m2_sb[:, o * P:(o + 1) * P],
                            h[:, lo:hi],
                            start=True,
                            stop=True,
                        )
                        ob = obuf.tile([P, hi - lo], dt_out, tag=f"ob{o}")
                        nc.vector.tensor_copy(ob[:], po[:])
                        eng = nc.scalar if o % 2 == 0 else nc.gpsimd
                        eng.dma_start(
                            outT_k[o, :, tok.start + lo:tok.start + hi], ob[:])

            # software pipeline: matmul1 of chunk c+1 is emitted before
            # matmul2 of chunk c, so the PE never stalls on the h copy
            hs = {0: emit_stage1(0)}
            for c in range(NCHUNK):
                if c + 1 < NCHUNK:
                    hs[c + 1] = emit_stage1(c + 1)
                emit_stage2(c, hs.pop(c))
    nc.compile()
    return nc


def _get_nc():
    global _NC
    if _NC is None:
        _NC = _build_bass()
    return _NC


def _host_gating(x, lora_A, lora_B, gate_w, gate_b):
    """Per-batch combined expert matrices M2[b] = sum of selected experts'
    score * SCALE * B rows (in the expert's row slot; rest zero)."""
    seg = np.asarray(x, np.float64).mean(axis=1)                    # [B, IN]
    logits = (seg @ np.asarray(gate_w, np.float64).T
              + np.asarray(gate_b, np.float64)) / TEMP              # [B, E]
    logits -= logits.max(axis=-1, keepdims=True)
    p = np.exp(logits)
    p /= p.sum(axis=-1, keepdims=True)
    top = np.argsort(-p, axis=-1, kind="stable")[:, :TOPK]          # [B, K]

    m2_all = np.zeros((B, E, OUT), np.float32)
    bcol = np.asarray(lora_B, np.float64)[:, :, 0]                  # [E, OUT]
    for b in range(B):
        for e in top[b]:
            m2_all[b, e, :] = (p[b, e] * SCALE) * bcol[e]
    return m2_all


def kernel(x, lora_A, lora_B, gate_w, gate_b):
    import ml_dtypes
    np_mm = np.float32 if MM_DTYPE != "bf16" else ml_dtypes.bfloat16

    x = np.ascontiguousarray(np.asarray(x, np.float32))
    lora_A = np.asarray(lora_A, np.float32)
    lora_B = np.asarray(lora_B, np.float32)

    m2_all = _host_gating(x, lora_A, lora_B, gate_w, gate_b)

    # aT[p, k*E+e] = lora_A[e, 0, k*128+p]  (replicated on all cores)
    a_mat = lora_A[:, 0, :]                                          # [E, IN]
    aT = np.ascontiguousarray(
        a_mat.T.reshape(KT, P, E).transpose(1, 0, 2).reshape(P, KT * E)
    ).astype(np_mm)

    xr = x.reshape(N_CORES, T, IN)
    in_maps = []
    for c in range(N_CORES):
        in_maps.append({
            "xT": np.ascontiguousarray(xr[c].T).astype(np_mm),       # [IN, T]
            "aT": aT,
            "m2": m2_all[c // 2].astype(np_mm),
        })

    res = run_bass_kernel_spmd(_get_nc(), in_maps, core_ids=list(range(N_CORES)))

    out = np.empty((N_CORES, T, OUT), np.float32)
    for c in range(N_CORES):
        out[c] = res.results[c]["outT"].T.astype(np.float32)
    return out.reshape(B, S, OUT)

